# revision 11
# baseline (speedup 1.0000x reference)
"""Trainium2 Bass kernel for nn_BimodalCrossAttentionBlock.

Math: seq-len-1 multihead cross attention => softmax over a single key is
identically 1, so MHA(x_q, x_kv) collapses to out_proj(v_proj(x_kv)) and the
two projections fold into one matrix Wc = out_w @ in_w[2D:] (Q/K projections
and num_heads are dead).  The block then is:
  graph_res = LN(graph + seq @ Wc_s2g.T + bc_s2g)     (gn1)
  seq_res   = LN(seq + graph @ Wc_g2s.T + bc_g2s)     (sn1)
  seq_out   = LN(seq_res + FFN_seq(seq_res))          (sn2)
  graph_out = LN(graph_res + FFN_gr(graph_res))       (gn2)

Sharding: modality-split data parallel.  Cores 0-3 compute seq_out for 8192
rows each; cores 4-7 compute graph_out for 8192 rows each.  Each core then
needs only ONE modality's weights (folded Wc 2MB + FFN w1/w2 16MB fp16),
which fits in SBUF alongside working tiles, so the whole block runs as one
fused, software-pipelined loop: attention matmul -> +residual -> LN1 ->
PE transpose -> FFN(w1/gelu/w2) -> +residual -> LN2 -> out.  No DRAM
round-trip for intermediates and no phase barriers, keeping the PE
continuously busy (TRN2 PE p-states make idle gaps extra costly).  The
kv-side operand is pre-transposed on the host (free) so the PE does no
fp32 input transposes.  Matmuls run fp16 with fp32 PSUM accumulation;
LayerNorm in fp32 (rsqrt via Newton iteration on DVE).
"""
import numpy as np

import concourse.bass as bass
import concourse.bacc as bacc
import concourse.tile as tile
import concourse.mybir as mybir
from concourse.bass_utils import run_bass_kernel_spmd
from concourse.masks import make_identity

F16 = mybir.dt.float16
F32 = mybir.dt.float32
U32 = mybir.dt.uint32
AF = mybir.ActivationFunctionType
ALU = mybir.AluOpType

N_CORES = 8
B_FULL = 32768
D = 1024
HID = 4096
R2 = B_FULL // 4      # rows per core (modality-split: 4 cores per modality)
NB = R2 // 256        # 256-row blocks per core
EPS = 1e-5
MAGIC = 0x5F3759DF

_cache = {}


def _ln_tail(nc, work, magic, x2, out_tile, lng_bc, lnb_bc):
    """LayerNorm of x2 [128, D] f32 -> out_tile; stats + rsqrt all on DVE."""
    stats = work.tile([128, 2, 6], F32, tag="lnstats")
    mv = work.tile([128, 2], F32, tag="lnmv")
    nc.vector.bn_stats(out=stats[:, 0, :], in_=x2[:, 0:512])
    nc.vector.bn_stats(out=stats[:, 1, :], in_=x2[:, 512:1024])
    nc.vector.bn_aggr(out=mv, in_=stats)
    v = work.tile([128, 1], F32, tag="lnv")
    nc.vector.tensor_scalar(out=v, in0=mv[:, 1:2], scalar1=EPS, scalar2=None,
                            op0=ALU.add)
    y = work.tile([128, 1], F32, tag="lny")
    t = work.tile([128, 1], F32, tag="lnt")
    nc.vector.tensor_scalar(out=y.bitcast(U32), in0=v.bitcast(U32), scalar1=1,
                            scalar2=None, op0=ALU.logical_shift_right)
    nc.vector.tensor_tensor(out=y.bitcast(U32), in0=magic, in1=y.bitcast(U32),
                            op=ALU.subtract)
    for _ in range(3):
        nc.vector.tensor_mul(out=t, in0=y, in1=y)
        nc.vector.tensor_mul(out=t, in0=t, in1=v)
        nc.vector.tensor_scalar(out=t, in0=t, scalar1=-0.5, scalar2=1.5,
                                op0=ALU.mult, op1=ALU.add)
        nc.vector.tensor_mul(out=y, in0=y, in1=t)
    if lng_bc is None and lnb_bc is None:
        nc.vector.tensor_scalar(out=out_tile, in0=x2, scalar1=mv[:, 0:1],
                                scalar2=y, op0=ALU.subtract, op1=ALU.mult)
    else:
        tmp = work.tile([128, 1024], F32, tag="lntmp")
        nc.vector.tensor_scalar(out=tmp, in0=x2, scalar1=mv[:, 0:1],
                                scalar2=y, op0=ALU.subtract, op1=ALU.mult)
        if lng_bc is not None:
            nc.vector.tensor_mul(out=tmp, in0=tmp, in1=lng_bc)
        if lnb_bc is not None:
            nc.vector.tensor_add(out=out_tile, in0=tmp, in1=lnb_bc)
        else:
            nc.vector.tensor_copy(out=out_tile, in_=tmp)


def _bcast_param(nc, pool, dram_ap, n, tag):
    t = pool.tile([128, n], F32, tag=tag)
    src = bass.AP(tensor=dram_ap.tensor, offset=dram_ap.offset,
                  ap=[[0, 128]] + dram_ap.ap)
    nc.gpsimd.dma_start(out=t, in_=src)
    return t


def _build(flags):
    fl = lambda k: bool(flags.get(k, False))
    nc = bacc.Bacc("TRN2", target_bir_lowering=False, debug=False,
                   num_devices=N_CORES)

    xq_d = nc.declare_dram_parameter("xq", [R2, D], F16, isOutput=False)
    kvt_d = nc.declare_dram_parameter("kvt", [128, 8, R2], F16, isOutput=False)
    wc_d = nc.declare_dram_parameter("wc", [128, 8, D], F16, isOutput=False)
    w1_d = nc.declare_dram_parameter("w1", [128, 8, HID], F16, isOutput=False)
    w2_d = nc.declare_dram_parameter("w2", [128, 32, D], F16, isOutput=False)
    opt = {}
    for nm, shape, dt in [("bc", [1, D], F16), ("b1", [128, 32], F32),
                          ("b2", [1, D], F16),
                          ("ln1_g", [D], F32), ("ln1_b", [D], F32),
                          ("ln2_g", [D], F32), ("ln2_b", [D], F32)]:
        if fl(nm):
            opt[nm] = nc.declare_dram_parameter(nm, shape, dt, isOutput=False)
    out_d = nc.declare_dram_parameter("out", [R2, D], F32, isOutput=True)

    with tile.TileContext(nc) as tc:
        with tc.tile_pool(name="singles", bufs=1) as singles, \
             tc.tile_pool(name="work", bufs=2) as work, \
             tc.tile_pool(name="lnw", bufs=4) as lnw, \
             tc.tile_pool(name="x2p", bufs=3) as x2p, \
             tc.tile_pool(name="hgp", bufs=8) as hgp, \
             tc.tile_pool(name="psA", bufs=4, space="PSUM") as psA, \
             tc.tile_pool(name="pso", bufs=4, space="PSUM") as pso:

            state = {}

            # ---- block-0 input DMAs first so attention can start early ----
            def load_block(i):
                if i in state:
                    return
                row = i * 256
                xq_t = work.tile([128, 2, D], F16, tag="xq")
                nc.sync.dma_start(
                    out=xq_t,
                    in_=xq_d[row:row + 256, :].rearrange("(s p) n -> p s n", p=128))
                kv_t = work.tile([128, 8, 256], F16, tag="kvt")
                nc.sync.dma_start(out=kv_t, in_=kvt_d[:, :, row:row + 256])
                state[i] = {"xq": xq_t, "kv": kv_t}

            # block 0: interleave kv chunks with wc chunks so the first
            # attention matmul (needs kv chunk kt + wc chunk kt) starts after
            # ~300KB of DMA instead of the full 2.5MB; xq deferred (only
            # needed for the residual add after the matmuls).
            wc_sb = singles.tile([128, 8, D], F16)
            kv0_t = work.tile([128, 8, 256], F16, tag="kvt")
            for kt in range(8):
                nc.sync.dma_start(out=kv0_t[:, kt, :], in_=kvt_d[:, kt, 0:256])
                nc.sync.dma_start(out=wc_sb[:, kt, :], in_=wc_d[:, kt, :])
            xq0_t = work.tile([128, 2, D], F16, tag="xq")
            nc.sync.dma_start(
                out=xq0_t,
                in_=xq_d[0:256, :].rearrange("(s p) n -> p s n", p=128))
            state[0] = {"xq": xq0_t, "kv": kv0_t}
            magic = singles.tile([128, 1], U32)
            nc.vector.memset(magic, MAGIC)
            ones16 = None
            bc_sb = None
            if fl("bc"):
                ones16 = singles.tile([1, 128], F16)
                nc.vector.memset(ones16, 1.0)
                bc_sb = singles.tile([1, D], F16)
                nc.sync.dma_start(out=bc_sb, in_=opt["bc"][:, :])
            ln_bcs = {}
            for nm in ("ln1_g", "ln1_b", "ln2_g", "ln2_b"):
                if nm in opt:
                    ln_bcs[nm] = _bcast_param(nc, singles, opt[nm].ap(), D, nm)

            # w1/w2 loaded in deadline order: ffn(0) consumes w1 quarter
            # ht//8 and w2 group ht//4 sequentially, so emit chunks in the
            # order the ht loop will first touch them.
            w1_sb = singles.tile([128, 8, HID], F16)
            w2_sb = singles.tile([128, 32, D], F16)

            # w1 streams on the sync HWDGE queue, w2 on the scalar HWDGE
            # queue (each engine has its own hardware queue, so the two
            # streams transfer in parallel during warmup).  The block-0
            # transpose DMA is emitted before the w2 loads (see below) so
            # rT(0) isn't head-of-line blocked behind 8MB of weights.
            def w1q(q):
                for kt in range(8):
                    nc.sync.dma_start(
                        out=w1_sb[:, kt, q * 1024:(q + 1) * 1024],
                        in_=w1_d[:, kt, q * 1024:(q + 1) * 1024])

            def w2g(g):
                nc.scalar.dma_start(out=w2_sb[:, g * 4:(g + 1) * 4, :],
                                    in_=w2_d[:, g * 4:(g + 1) * 4, :])
            b1_sb = None
            if fl("b1"):
                b1_sb = singles.tile([128, 32], F32)
                nc.sync.dma_start(out=b1_sb, in_=opt["b1"][:, :])
            b2_sb = None
            if fl("b2"):
                if ones16 is None:
                    ones16 = singles.tile([1, 128], F16)
                    nc.vector.memset(ones16, 1.0)
                b2_sb = singles.tile([1, D], F16)
                nc.sync.dma_start(out=b2_sb, in_=opt["b2"][:, :])

            # ---------------- pipelined emission ----------------
            def emit_attn(i):
                st = state[i]
                xq_t, kv_t = st["xq"], st["kv"]
                res16 = work.tile([128, 2, D], F16, tag="res16")
                for sub in range(2):
                    pa = [psA.tile([128, 512], F32, tag="acc",
                                   name=f"pa{i}_{sub}_{h}") for h in range(2)]
                    for h in range(2):
                        nsl = slice(h * 512, (h + 1) * 512)
                        for kt in range(8):
                            nc.tensor.matmul(pa[h],
                                             lhsT=kv_t[:, kt, sub * 128:(sub + 1) * 128],
                                             rhs=wc_sb[:, kt, nsl],
                                             start=(kt == 0),
                                             stop=(kt == 7 and bc_sb is None))
                        if bc_sb is not None:
                            nc.tensor.matmul(pa[h], lhsT=ones16,
                                             rhs=bc_sb[:, nsl],
                                             start=False, stop=True)
                    x = work.tile([128, D], F32, tag="x")
                    nc.vector.tensor_add(out=x[:, 0:512],
                                         in0=xq_t[:, sub, 0:512], in1=pa[0])
                    nc.vector.tensor_add(out=x[:, 512:1024],
                                         in0=xq_t[:, sub, 512:1024], in1=pa[1])
                    _ln_tail(nc, lnw, magic, x, res16[:, sub, :],
                             ln_bcs.get("ln1_g"), ln_bcs.get("ln1_b"))
                st["res16"] = res16

            def emit_tp(i):
                # res16^T via the DMA crossbar (scalar-engine HWDGE queue):
                # out[p, kt, row] = res16[row, kt*128+p].  Keeps the PE free
                # of transpose work entirely.
                res16 = state[i]["res16"]
                rT = work.tile([128, 8, 256], F16, tag="rT")
                for sub in range(2):
                    nc.scalar.dma_start(
                        out=rT[:, :, sub * 128:(sub + 1) * 128],
                        in_=res16[:, sub, :], transpose=True)
                state[i]["rT"] = rT

            def emit_ffn(i):
                rT = state[i]["rT"]
                ops = [pso.tile([128, 512], F32, tag="ops",
                                name=f"ops{i}_{h}") for h in range(4)]
                for ht in range(32):
                    hps = psA.tile([128, 512], F32, tag="acc",
                                   name=f"hps{i}_{ht}")
                    for kt in range(8):
                        nc.tensor.matmul(hps[:, 0:256],
                                         lhsT=w1_sb[:, kt, ht * 128:(ht + 1) * 128],
                                         rhs=rT[:, kt, :],
                                         start=(kt == 0), stop=(kt == 7))
                    hg = hgp.tile([128, 256], F16, tag="hg")
                    if b1_sb is not None:
                        nc.scalar.activation(out=hg, in_=hps[:, 0:256],
                                             func=AF.Gelu,
                                             bias=b1_sb[:, ht:ht + 1],
                                             scale=1.0, alpha=0.0)
                    else:
                        nc.scalar.activation(out=hg, in_=hps[:, 0:256],
                                             func=AF.Gelu)
                    for bs in range(2):
                        for nh in range(2):
                            nc.tensor.matmul(
                                ops[bs * 2 + nh],
                                lhsT=hg[:, bs * 128:(bs + 1) * 128],
                                rhs=w2_sb[:, ht, nh * 512:(nh + 1) * 512],
                                start=(ht == 0),
                                stop=(ht == 31 and b2_sb is None))
                if b2_sb is not None:
                    for bs in range(2):
                        for nh in range(2):
                            nc.tensor.matmul(ops[bs * 2 + nh], lhsT=ones16,
                                             rhs=b2_sb[:, nh * 512:(nh + 1) * 512],
                                             start=False, stop=True)
                state[i]["ops"] = ops

            def emit_out(i):
                res16 = state[i]["res16"]
                ops = state[i]["ops"]
                row = i * 256
                for bs in range(2):
                    x2 = x2p.tile([128, D], F32, tag="x2")
                    nc.vector.tensor_add(out=x2[:, 0:512],
                                         in0=res16[:, bs, 0:512],
                                         in1=ops[bs * 2 + 0])
                    nc.vector.tensor_add(out=x2[:, 512:1024],
                                         in0=res16[:, bs, 512:1024],
                                         in1=ops[bs * 2 + 1])
                    _ln_tail(nc, lnw, magic, x2, x2,
                             ln_bcs.get("ln2_g"), ln_bcs.get("ln2_b"))
                    nc.sync.dma_start(
                        out=out_d[row + bs * 128:row + bs * 128 + 128, :],
                        in_=x2)
                del state[i]

            # last block runs as two 128-row FFN passes so the end-of-kernel
            # drain (adds+LN2+DMA after the final w2 matmul) covers 128 rows
            # instead of 256; the first half's output path overlaps the
            # second half's FFN.
            def emit_ffn_half(i, half):
                rT = state[i]["rT"]
                ops = [pso.tile([128, 512], F32, tag="ops",
                                name=f"opsh{i}_{half}_{h}") for h in range(2)]
                for ht in range(32):
                    hps = psA.tile([128, 512], F32, tag="acc",
                                   name=f"hpsh{i}_{half}_{ht}")
                    for kt in range(8):
                        nc.tensor.matmul(hps[:, 0:128],
                                         lhsT=w1_sb[:, kt, ht * 128:(ht + 1) * 128],
                                         rhs=rT[:, kt, half * 128:(half + 1) * 128],
                                         start=(kt == 0), stop=(kt == 7))
                    hg = hgp.tile([128, 256], F16, tag="hg")
                    if b1_sb is not None:
                        nc.scalar.activation(out=hg[:, 0:128], in_=hps[:, 0:128],
                                             func=AF.Gelu,
                                             bias=b1_sb[:, ht:ht + 1],
                                             scale=1.0, alpha=0.0)
                    else:
                        nc.scalar.activation(out=hg[:, 0:128], in_=hps[:, 0:128],
                                             func=AF.Gelu)
                    for nh in range(2):
                        nc.tensor.matmul(ops[nh], lhsT=hg[:, 0:128],
                                         rhs=w2_sb[:, ht, nh * 512:(nh + 1) * 512],
                                         start=(ht == 0),
                                         stop=(ht == 31 and b2_sb is None))
                if b2_sb is not None:
                    for nh in range(2):
                        nc.tensor.matmul(ops[nh], lhsT=ones16,
                                         rhs=b2_sb[:, nh * 512:(nh + 1) * 512],
                                         start=False, stop=True)
                return ops

            def emit_out_half(i, half, ops):
                res16 = state[i]["res16"]
                row = i * 256 + half * 128
                x2 = x2p.tile([128, D], F32, tag="x2")
                nc.vector.tensor_add(out=x2[:, 0:512],
                                     in0=res16[:, half, 0:512], in1=ops[0])
                nc.vector.tensor_add(out=x2[:, 512:1024],
                                     in0=res16[:, half, 512:1024], in1=ops[1])
                _ln_tail(nc, lnw, magic, x2, x2,
                         ln_bcs.get("ln2_g"), ln_bcs.get("ln2_b"))
                nc.sync.dma_start(out=out_d[row:row + 128, :], in_=x2)

            emit_attn(0)
            emit_tp(0)
            w1q(0)
            w2g(0)
            load_block(1)
            w2g(1)
            w1q(1)
            w2g(2)
            w2g(3)
            w1q(2)
            w2g(4)
            w2g(5)
            w1q(3)
            w2g(6)
            w2g(7)
            for i in range(NB):
                if i + 1 < NB:
                    load_block(i + 1)
                    emit_attn(i + 1)
                if i == NB - 1:
                    opsA = emit_ffn_half(i, 0)
                    emit_out_half(i, 0, opsA)
                    opsB = emit_ffn_half(i, 1)
                    emit_out_half(i, 1, opsB)
                    del state[i]
                else:
                    emit_ffn(i)
                    emit_tp(i + 1)
                    emit_out(i)

    nc.compile()
    return nc


def _host_prep(inputs):
    f = lambda k: np.asarray(inputs[k])
    flags = {}

    def fold(pfx):
        in_w = f(f"{pfx}_in_w").astype(np.float64)
        in_b = f(f"{pfx}_in_b").astype(np.float64)
        out_w = f(f"{pfx}_out_w").astype(np.float64)
        out_b = f(f"{pfx}_out_b").astype(np.float64)
        Wc = out_w @ in_w[2 * D:]
        bc = in_b[2 * D:] @ out_w.T + out_b
        return Wc, bc

    Wcs, bcs = fold("s2g")   # kv = seq, updates graph
    Wcg, bcg = fold("g2s")   # kv = graph, updates seq

    def rhs_tiles(W, kt):  # W [n, d_in] -> [128, kt, n] f16 tiles of W.T
        return np.ascontiguousarray(
            W.T.reshape(kt, 128, -1).transpose(1, 0, 2)).astype(np.float16)

    def t_tiles(X):  # X [B, D] -> [128, 8, B] f16 tiles of X.T
        return np.ascontiguousarray(
            X.T.reshape(8, 128, -1).transpose(1, 0, 2)).astype(np.float16)

    seq = f("seq_emb").astype(np.float32)
    graph = f("graph_emb").astype(np.float32)
    seqT = t_tiles(seq)
    graphT = t_tiles(graph)
    seq16 = seq.astype(np.float16)
    graph16 = graph.astype(np.float16)

    # flags are the union over both modalities (one SPMD program for all
    # cores); zero/identity values are passed where a modality's param is
    # trivial.
    flags_probe = {
        "bc": np.any(bcs != 0) or np.any(bcg != 0),
        "b1": np.any(f("seq_b1") != 0) or np.any(f("gr_b1") != 0),
        "b2": np.any(f("seq_b2") != 0) or np.any(f("gr_b2") != 0),
        "ln1_g": np.any(f("sn1_g") != 1) or np.any(f("gn1_g") != 1),
        "ln1_b": np.any(f("sn1_b") != 0) or np.any(f("gn1_b") != 0),
        "ln2_g": np.any(f("sn2_g") != 1) or np.any(f("gn2_g") != 1),
        "ln2_b": np.any(f("sn2_b") != 0) or np.any(f("gn2_b") != 0),
    }
    for k, v in flags_probe.items():
        if v:
            flags[k] = True

    def modality_map(wc, bc, w1, b1, w2, b2, ln1g, ln1b, ln2g, ln2b):
        m = {"wc": rhs_tiles(wc, 8), "w1": rhs_tiles(w1, 8),
             "w2": rhs_tiles(w2, 32)}
        if "bc" in flags:
            m["bc"] = bc.astype(np.float16).reshape(1, D)
        if "b1" in flags:
            m["b1"] = np.ascontiguousarray(
                b1.reshape(32, 128).T).astype(np.float32)
        if "b2" in flags:
            m["b2"] = b2.astype(np.float16).reshape(1, D)
        for nm, v, dflt in (("ln1_g", ln1g, 1.0), ("ln1_b", ln1b, 0.0),
                            ("ln2_g", ln2g, 1.0), ("ln2_b", ln2b, 0.0)):
            if nm in flags:
                m[nm] = np.asarray(v, dtype=np.float32)
        return m

    # seq cores: xq = seq, kv = graph, wc = Wcg (g2s), FFN = seq_*
    wm_s = modality_map(Wcg, bcg, f("seq_w1"), f("seq_b1"), f("seq_w2"),
                        f("seq_b2"), f("sn1_g"), f("sn1_b"), f("sn2_g"),
                        f("sn2_b"))
    # graph cores: xq = graph, kv = seq, wc = Wcs (s2g), FFN = gr_*
    wm_g = modality_map(Wcs, bcs, f("gr_w1"), f("gr_b1"), f("gr_w2"),
                        f("gr_b2"), f("gn1_g"), f("gn1_b"), f("gn2_g"),
                        f("gn2_b"))

    in_maps = []
    for i in range(N_CORES):
        if i < 4:
            m = dict(wm_s)
            sl = slice(i * R2, (i + 1) * R2)
            m["xq"] = np.ascontiguousarray(seq16[sl])
            m["kvt"] = np.ascontiguousarray(graphT[:, :, sl])
        else:
            m = dict(wm_g)
            sl = slice((i - 4) * R2, (i - 3) * R2)
            m["xq"] = np.ascontiguousarray(graph16[sl])
            m["kvt"] = np.ascontiguousarray(seqT[:, :, sl])
        in_maps.append(m)
    return in_maps, flags


def kernel(**inputs):
    in_maps, flags = _host_prep(inputs)
    key = tuple(sorted(flags.items()))
    if key not in _cache:
        _cache[key] = _build(flags)
    nc = _cache[key]
    res = run_bass_kernel_spmd(nc, in_maps, core_ids=list(range(N_CORES)))
    seq_out = np.concatenate([res.results[i]["out"] for i in range(4)], axis=0)
    graph_out = np.concatenate([res.results[i]["out"] for i in range(4, 8)],
                               axis=0)
    return (seq_out, graph_out)


# revision 13
# speedup vs baseline: 1.1847x; 1.1847x over previous
"""Trainium2 Bass kernel for nn_BimodalCrossAttentionBlock.

Math: seq-len-1 multihead cross attention => softmax over a single key is
identically 1, so MHA(x_q, x_kv) collapses to out_proj(v_proj(x_kv)) and the
two projections fold into one matrix Wc = out_w @ in_w[2D:] (Q/K projections
and num_heads are dead).  The block then is:
  graph_res = LN(graph + seq @ Wc_s2g.T + bc_s2g)     (gn1)
  seq_res   = LN(seq + graph @ Wc_g2s.T + bc_g2s)     (sn1)
  seq_out   = LN(seq_res + FFN_seq(seq_res))          (sn2)
  graph_out = LN(graph_res + FFN_gr(graph_res))       (gn2)

Sharding: modality-split data parallel.  Cores 0-3 compute seq_out for 8192
rows each; cores 4-7 compute graph_out for 8192 rows each.  Each core then
needs only ONE modality's weights (folded Wc 2MB + FFN w1/w2 16MB fp16),
which fits in SBUF alongside working tiles, so the whole block runs as one
fused, software-pipelined loop: attention matmul -> +residual -> LN1 ->
PE transpose -> FFN(w1/gelu/w2) -> +residual -> LN2 -> out.  No DRAM
round-trip for intermediates and no phase barriers, keeping the PE
continuously busy (TRN2 PE p-states make idle gaps extra costly).  The
kv-side operand is pre-transposed on the host (free) so the PE does no
fp32 input transposes.  Matmuls run fp16 with fp32 PSUM accumulation;
LayerNorm in fp32 (rsqrt via Newton iteration on DVE).
"""
import numpy as np

import concourse.bass as bass
import concourse.bacc as bacc
import concourse.tile as tile
import concourse.mybir as mybir
from concourse.bass_utils import run_bass_kernel_spmd
from concourse.masks import make_identity

F16 = mybir.dt.float16
F32 = mybir.dt.float32
U32 = mybir.dt.uint32
AF = mybir.ActivationFunctionType
ALU = mybir.AluOpType

N_CORES = 8
B_FULL = 32768
D = 1024
HID = 4096
R2 = B_FULL // 4      # rows per core (modality-split: 4 cores per modality)
NB = R2 // 256        # 256-row blocks per core
EPS = 1e-5
MAGIC = 0x5F3759DF

_cache = {}


def _ln_tail(nc, work, magic, x2, out_tile, lng_bc, lnb_bc):
    """LayerNorm of x2 [128, D] f32 -> out_tile; stats + rsqrt all on DVE."""
    stats = work.tile([128, 2, 6], F32, tag="lnstats")
    mv = work.tile([128, 2], F32, tag="lnmv")
    nc.vector.bn_stats(out=stats[:, 0, :], in_=x2[:, 0:512])
    nc.vector.bn_stats(out=stats[:, 1, :], in_=x2[:, 512:1024])
    nc.vector.bn_aggr(out=mv, in_=stats)
    v = work.tile([128, 1], F32, tag="lnv")
    nc.vector.tensor_scalar(out=v, in0=mv[:, 1:2], scalar1=EPS, scalar2=None,
                            op0=ALU.add)
    y = work.tile([128, 1], F32, tag="lny")
    t = work.tile([128, 1], F32, tag="lnt")
    nc.vector.tensor_scalar(out=y.bitcast(U32), in0=v.bitcast(U32), scalar1=1,
                            scalar2=None, op0=ALU.logical_shift_right)
    nc.vector.tensor_tensor(out=y.bitcast(U32), in0=magic, in1=y.bitcast(U32),
                            op=ALU.subtract)
    for _ in range(3):
        nc.vector.tensor_mul(out=t, in0=y, in1=y)
        nc.vector.tensor_mul(out=t, in0=t, in1=v)
        nc.vector.tensor_scalar(out=t, in0=t, scalar1=-0.5, scalar2=1.5,
                                op0=ALU.mult, op1=ALU.add)
        nc.vector.tensor_mul(out=y, in0=y, in1=t)
    if lng_bc is None and lnb_bc is None:
        nc.vector.tensor_scalar(out=out_tile, in0=x2, scalar1=mv[:, 0:1],
                                scalar2=y, op0=ALU.subtract, op1=ALU.mult)
    else:
        tmp = work.tile([128, 1024], F32, tag="lntmp")
        nc.vector.tensor_scalar(out=tmp, in0=x2, scalar1=mv[:, 0:1],
                                scalar2=y, op0=ALU.subtract, op1=ALU.mult)
        if lng_bc is not None:
            nc.vector.tensor_mul(out=tmp, in0=tmp, in1=lng_bc)
        if lnb_bc is not None:
            nc.vector.tensor_add(out=out_tile, in0=tmp, in1=lnb_bc)
        else:
            nc.vector.tensor_copy(out=out_tile, in_=tmp)


def _bcast_param(nc, pool, dram_ap, n, tag):
    t = pool.tile([128, n], F32, tag=tag)
    src = bass.AP(tensor=dram_ap.tensor, offset=dram_ap.offset,
                  ap=[[0, 128]] + dram_ap.ap)
    nc.gpsimd.dma_start(out=t, in_=src)
    return t


def _build(flags):
    fl = lambda k: bool(flags.get(k, False))
    nc = bacc.Bacc("TRN2", target_bir_lowering=False, debug=False,
                   num_devices=N_CORES)

    xq_d = nc.declare_dram_parameter("xq", [R2, D], F16, isOutput=False)
    kvt_d = nc.declare_dram_parameter("kvt", [128, 8, R2], F16, isOutput=False)
    wc_d = nc.declare_dram_parameter("wc", [128, 8, D], F16, isOutput=False)
    w1_d = nc.declare_dram_parameter("w1", [128, 8, HID], F16, isOutput=False)
    w2_d = nc.declare_dram_parameter("w2", [128, 32, D], F16, isOutput=False)
    opt = {}
    for nm, shape, dt in [("bc", [1, D], F16), ("b1", [128, 32], F32),
                          ("b2", [1, D], F16),
                          ("ln1_g", [D], F32), ("ln1_b", [D], F32),
                          ("ln2_g", [D], F32), ("ln2_b", [D], F32)]:
        if fl(nm):
            opt[nm] = nc.declare_dram_parameter(nm, shape, dt, isOutput=False)
    out_d = nc.declare_dram_parameter("out", [R2, D], F32, isOutput=True)

    with tile.TileContext(nc) as tc:
        with tc.tile_pool(name="singles", bufs=1) as singles, \
             tc.tile_pool(name="work", bufs=2) as work, \
             tc.tile_pool(name="lnw", bufs=4) as lnw, \
             tc.tile_pool(name="x2p", bufs=3) as x2p, \
             tc.tile_pool(name="hgp", bufs=8) as hgp, \
             tc.tile_pool(name="psA", bufs=4, space="PSUM") as psA, \
             tc.tile_pool(name="pso", bufs=4, space="PSUM") as pso:

            state = {}

            # ---- block-0 input DMAs first so attention can start early ----
            def load_block(i):
                if i in state:
                    return
                row = i * 256
                xq_t = work.tile([128, 2, D], F16, tag="xq")
                nc.sync.dma_start(
                    out=xq_t,
                    in_=xq_d[row:row + 256, :].rearrange("(s p) n -> p s n", p=128))
                kv_t = work.tile([128, 8, 256], F16, tag="kvt")
                nc.sync.dma_start(out=kv_t, in_=kvt_d[:, :, row:row + 256])
                state[i] = {"xq": xq_t, "kv": kv_t}

            # block 0: interleave kv chunks with wc chunks so the first
            # attention matmul (needs kv chunk kt + wc chunk kt) starts after
            # ~300KB of DMA instead of the full 2.5MB; xq deferred (only
            # needed for the residual add after the matmuls).
            wc_sb = singles.tile([128, 8, D], F16)
            kv0_t = work.tile([128, 8, 256], F16, tag="kvt")
            for kt in range(8):
                nc.sync.dma_start(out=kv0_t[:, kt, :], in_=kvt_d[:, kt, 0:256])
                nc.sync.dma_start(out=wc_sb[:, kt, :], in_=wc_d[:, kt, :])
            xq0_t = work.tile([128, 2, D], F16, tag="xq")
            nc.sync.dma_start(
                out=xq0_t,
                in_=xq_d[0:256, :].rearrange("(s p) n -> p s n", p=128))
            state[0] = {"xq": xq0_t, "kv": kv0_t}
            ident16 = singles.tile([128, 128], F16)
            make_identity(nc, ident16)
            magic = singles.tile([128, 1], U32)
            nc.vector.memset(magic, MAGIC)
            ones16 = None
            bc_sb = None
            if fl("bc"):
                ones16 = singles.tile([1, 128], F16)
                nc.vector.memset(ones16, 1.0)
                bc_sb = singles.tile([1, D], F16)
                nc.sync.dma_start(out=bc_sb, in_=opt["bc"][:, :])
            ln_bcs = {}
            for nm in ("ln1_g", "ln1_b", "ln2_g", "ln2_b"):
                if nm in opt:
                    ln_bcs[nm] = _bcast_param(nc, singles, opt[nm].ap(), D, nm)

            # w1/w2 loaded in deadline order: ffn(0) consumes w1 quarter
            # ht//8 and w2 group ht//4 sequentially, so emit chunks in the
            # order the ht loop will first touch them.
            w1_sb = singles.tile([128, 8, HID], F16)
            w2_sb = singles.tile([128, 32, D], F16)

            # w1 streams on the sync HWDGE queue, w2 on the scalar HWDGE
            # queue (each engine has its own hardware queue, so the two
            # streams transfer in parallel during warmup).  The block-0
            # transpose DMA is emitted before the w2 loads (see below) so
            # rT(0) isn't head-of-line blocked behind 8MB of weights.
            def w1q(q):
                for kt in range(8):
                    nc.sync.dma_start(
                        out=w1_sb[:, kt, q * 1024:(q + 1) * 1024],
                        in_=w1_d[:, kt, q * 1024:(q + 1) * 1024])

            def w2g(g):
                nc.scalar.dma_start(out=w2_sb[:, g * 4:(g + 1) * 4, :],
                                    in_=w2_d[:, g * 4:(g + 1) * 4, :])
            b1_sb = None
            if fl("b1"):
                b1_sb = singles.tile([128, 32], F32)
                nc.sync.dma_start(out=b1_sb, in_=opt["b1"][:, :])
            b2_sb = None
            if fl("b2"):
                if ones16 is None:
                    ones16 = singles.tile([1, 128], F16)
                    nc.vector.memset(ones16, 1.0)
                b2_sb = singles.tile([1, D], F16)
                nc.sync.dma_start(out=b2_sb, in_=opt["b2"][:, :])

            # ---------------- pipelined emission ----------------
            def emit_attn(i):
                st = state[i]
                xq_t, kv_t = st["xq"], st["kv"]
                res16 = work.tile([128, 2, D], F16, tag="res16")
                for sub in range(2):
                    pa = [psA.tile([128, 512], F32, tag="acc",
                                   name=f"pa{i}_{sub}_{h}") for h in range(2)]
                    for h in range(2):
                        nsl = slice(h * 512, (h + 1) * 512)
                        for kt in range(8):
                            nc.tensor.matmul(pa[h],
                                             lhsT=kv_t[:, kt, sub * 128:(sub + 1) * 128],
                                             rhs=wc_sb[:, kt, nsl],
                                             start=(kt == 0),
                                             stop=(kt == 7 and bc_sb is None))
                        if bc_sb is not None:
                            nc.tensor.matmul(pa[h], lhsT=ones16,
                                             rhs=bc_sb[:, nsl],
                                             start=False, stop=True)
                    x = work.tile([128, D], F32, tag="x")
                    nc.vector.tensor_add(out=x[:, 0:512],
                                         in0=xq_t[:, sub, 0:512], in1=pa[0])
                    nc.vector.tensor_add(out=x[:, 512:1024],
                                         in0=xq_t[:, sub, 512:1024], in1=pa[1])
                    _ln_tail(nc, lnw, magic, x, res16[:, sub, :],
                             ln_bcs.get("ln1_g"), ln_bcs.get("ln1_b"))
                st["res16"] = res16

            def emit_tp(i):
                res16 = state[i]["res16"]
                rT = work.tile([128, 8, 256], F16, tag="rT")
                for sub in range(2):
                    for grp in range(2):
                        tp = psA.tile([128, 512], F32, tag="acc",
                                      name=f"tp{i}_{sub}_{grp}")
                        tp16 = tp.bitcast(F16)
                        for j in range(4):
                            kt = grp * 4 + j
                            nc.tensor.transpose(tp16[:, j * 128:(j + 1) * 128],
                                                res16[:, sub, kt * 128:(kt + 1) * 128],
                                                ident16)
                        nc.vector.tensor_copy(
                            out=rT[:, grp * 4:(grp + 1) * 4,
                                   sub * 128:(sub + 1) * 128],
                            in_=tp16[:, 0:512].rearrange("p (a b) -> p a b",
                                                         b=128))
                state[i]["rT"] = rT

            def emit_ffn(i):
                rT = state[i]["rT"]
                ops = [pso.tile([128, 512], F32, tag="ops",
                                name=f"ops{i}_{h}") for h in range(4)]
                for ht in range(32):
                    hps = psA.tile([128, 512], F32, tag="acc",
                                   name=f"hps{i}_{ht}")
                    for kt in range(8):
                        nc.tensor.matmul(hps[:, 0:256],
                                         lhsT=w1_sb[:, kt, ht * 128:(ht + 1) * 128],
                                         rhs=rT[:, kt, :],
                                         start=(kt == 0), stop=(kt == 7))
                    hg = hgp.tile([128, 256], F16, tag="hg")
                    if b1_sb is not None:
                        nc.scalar.activation(out=hg, in_=hps[:, 0:256],
                                             func=AF.Gelu,
                                             bias=b1_sb[:, ht:ht + 1],
                                             scale=1.0, alpha=0.0)
                    else:
                        nc.scalar.activation(out=hg, in_=hps[:, 0:256],
                                             func=AF.Gelu)
                    for bs in range(2):
                        for nh in range(2):
                            nc.tensor.matmul(
                                ops[bs * 2 + nh],
                                lhsT=hg[:, bs * 128:(bs + 1) * 128],
                                rhs=w2_sb[:, ht, nh * 512:(nh + 1) * 512],
                                start=(ht == 0),
                                stop=(ht == 31 and b2_sb is None))
                if b2_sb is not None:
                    for bs in range(2):
                        for nh in range(2):
                            nc.tensor.matmul(ops[bs * 2 + nh], lhsT=ones16,
                                             rhs=b2_sb[:, nh * 512:(nh + 1) * 512],
                                             start=False, stop=True)
                state[i]["ops"] = ops

            def emit_out(i):
                res16 = state[i]["res16"]
                ops = state[i]["ops"]
                row = i * 256
                for bs in range(2):
                    x2 = x2p.tile([128, D], F32, tag="x2")
                    nc.vector.tensor_add(out=x2[:, 0:512],
                                         in0=res16[:, bs, 0:512],
                                         in1=ops[bs * 2 + 0])
                    nc.vector.tensor_add(out=x2[:, 512:1024],
                                         in0=res16[:, bs, 512:1024],
                                         in1=ops[bs * 2 + 1])
                    _ln_tail(nc, lnw, magic, x2, x2,
                             ln_bcs.get("ln2_g"), ln_bcs.get("ln2_b"))
                    nc.sync.dma_start(
                        out=out_d[row + bs * 128:row + bs * 128 + 128, :],
                        in_=x2)
                del state[i]

            # last block runs as two 128-row FFN passes so the end-of-kernel
            # drain (adds+LN2+DMA after the final w2 matmul) covers 128 rows
            # instead of 256; the first half's output path overlaps the
            # second half's FFN.
            def emit_ffn_half(i, half):
                rT = state[i]["rT"]
                ops = [pso.tile([128, 512], F32, tag="ops",
                                name=f"opsh{i}_{half}_{h}") for h in range(2)]
                for ht in range(32):
                    hps = psA.tile([128, 512], F32, tag="acc",
                                   name=f"hpsh{i}_{half}_{ht}")
                    for kt in range(8):
                        nc.tensor.matmul(hps[:, 0:128],
                                         lhsT=w1_sb[:, kt, ht * 128:(ht + 1) * 128],
                                         rhs=rT[:, kt, half * 128:(half + 1) * 128],
                                         start=(kt == 0), stop=(kt == 7))
                    hg = hgp.tile([128, 256], F16, tag="hg")
                    if b1_sb is not None:
                        nc.scalar.activation(out=hg[:, 0:128], in_=hps[:, 0:128],
                                             func=AF.Gelu,
                                             bias=b1_sb[:, ht:ht + 1],
                                             scale=1.0, alpha=0.0)
                    else:
                        nc.scalar.activation(out=hg[:, 0:128], in_=hps[:, 0:128],
                                             func=AF.Gelu)
                    for nh in range(2):
                        nc.tensor.matmul(ops[nh], lhsT=hg[:, 0:128],
                                         rhs=w2_sb[:, ht, nh * 512:(nh + 1) * 512],
                                         start=(ht == 0),
                                         stop=(ht == 31 and b2_sb is None))
                if b2_sb is not None:
                    for nh in range(2):
                        nc.tensor.matmul(ops[nh], lhsT=ones16,
                                         rhs=b2_sb[:, nh * 512:(nh + 1) * 512],
                                         start=False, stop=True)
                return ops

            def emit_out_half(i, half, ops):
                res16 = state[i]["res16"]
                row = i * 256 + half * 128
                x2 = x2p.tile([128, D], F32, tag="x2")
                nc.vector.tensor_add(out=x2[:, 0:512],
                                     in0=res16[:, half, 0:512], in1=ops[0])
                nc.vector.tensor_add(out=x2[:, 512:1024],
                                     in0=res16[:, half, 512:1024], in1=ops[1])
                _ln_tail(nc, lnw, magic, x2, x2,
                         ln_bcs.get("ln2_g"), ln_bcs.get("ln2_b"))
                nc.sync.dma_start(out=out_d[row:row + 128, :], in_=x2)

            emit_attn(0)
            emit_tp(0)
            w1q(0)
            w2g(0)
            load_block(1)
            w2g(1)
            w1q(1)
            w2g(2)
            w2g(3)
            w1q(2)
            w2g(4)
            w2g(5)
            w1q(3)
            w2g(6)
            w2g(7)
            for i in range(NB):
                if i + 1 < NB:
                    load_block(i + 1)
                    emit_attn(i + 1)
                if i == NB - 1:
                    opsA = emit_ffn_half(i, 0)
                    emit_out_half(i, 0, opsA)
                    opsB = emit_ffn_half(i, 1)
                    emit_out_half(i, 1, opsB)
                    del state[i]
                else:
                    emit_ffn(i)
                    emit_tp(i + 1)
                    emit_out(i)

    nc.compile()
    return nc


def _host_prep(inputs):
    f = lambda k: np.asarray(inputs[k])
    flags = {}

    def fold(pfx):
        in_w = f(f"{pfx}_in_w").astype(np.float64)
        in_b = f(f"{pfx}_in_b").astype(np.float64)
        out_w = f(f"{pfx}_out_w").astype(np.float64)
        out_b = f(f"{pfx}_out_b").astype(np.float64)
        Wc = out_w @ in_w[2 * D:]
        bc = in_b[2 * D:] @ out_w.T + out_b
        return Wc, bc

    Wcs, bcs = fold("s2g")   # kv = seq, updates graph
    Wcg, bcg = fold("g2s")   # kv = graph, updates seq

    def rhs_tiles(W, kt):  # W [n, d_in] -> [128, kt, n] f16 tiles of W.T
        return np.ascontiguousarray(
            W.T.reshape(kt, 128, -1).transpose(1, 0, 2)).astype(np.float16)

    def t_tiles(X):  # X [B, D] -> [128, 8, B] f16 tiles of X.T
        return np.ascontiguousarray(
            X.T.reshape(8, 128, -1).transpose(1, 0, 2)).astype(np.float16)

    seq = f("seq_emb").astype(np.float32)
    graph = f("graph_emb").astype(np.float32)
    seqT = t_tiles(seq)
    graphT = t_tiles(graph)
    seq16 = seq.astype(np.float16)
    graph16 = graph.astype(np.float16)

    # flags are the union over both modalities (one SPMD program for all
    # cores); zero/identity values are passed where a modality's param is
    # trivial.
    flags_probe = {
        "bc": np.any(bcs != 0) or np.any(bcg != 0),
        "b1": np.any(f("seq_b1") != 0) or np.any(f("gr_b1") != 0),
        "b2": np.any(f("seq_b2") != 0) or np.any(f("gr_b2") != 0),
        "ln1_g": np.any(f("sn1_g") != 1) or np.any(f("gn1_g") != 1),
        "ln1_b": np.any(f("sn1_b") != 0) or np.any(f("gn1_b") != 0),
        "ln2_g": np.any(f("sn2_g") != 1) or np.any(f("gn2_g") != 1),
        "ln2_b": np.any(f("sn2_b") != 0) or np.any(f("gn2_b") != 0),
    }
    for k, v in flags_probe.items():
        if v:
            flags[k] = True

    def modality_map(wc, bc, w1, b1, w2, b2, ln1g, ln1b, ln2g, ln2b):
        m = {"wc": rhs_tiles(wc, 8), "w1": rhs_tiles(w1, 8),
             "w2": rhs_tiles(w2, 32)}
        if "bc" in flags:
            m["bc"] = bc.astype(np.float16).reshape(1, D)
        if "b1" in flags:
            m["b1"] = np.ascontiguousarray(
                b1.reshape(32, 128).T).astype(np.float32)
        if "b2" in flags:
            m["b2"] = b2.astype(np.float16).reshape(1, D)
        for nm, v, dflt in (("ln1_g", ln1g, 1.0), ("ln1_b", ln1b, 0.0),
                            ("ln2_g", ln2g, 1.0), ("ln2_b", ln2b, 0.0)):
            if nm in flags:
                m[nm] = np.asarray(v, dtype=np.float32)
        return m

    # seq cores: xq = seq, kv = graph, wc = Wcg (g2s), FFN = seq_*
    wm_s = modality_map(Wcg, bcg, f("seq_w1"), f("seq_b1"), f("seq_w2"),
                        f("seq_b2"), f("sn1_g"), f("sn1_b"), f("sn2_g"),
                        f("sn2_b"))
    # graph cores: xq = graph, kv = seq, wc = Wcs (s2g), FFN = gr_*
    wm_g = modality_map(Wcs, bcs, f("gr_w1"), f("gr_b1"), f("gr_w2"),
                        f("gr_b2"), f("gn1_g"), f("gn1_b"), f("gn2_g"),
                        f("gn2_b"))

    in_maps = []
    for i in range(N_CORES):
        if i < 4:
            m = dict(wm_s)
            sl = slice(i * R2, (i + 1) * R2)
            m["xq"] = np.ascontiguousarray(seq16[sl])
            m["kvt"] = np.ascontiguousarray(graphT[:, :, sl])
        else:
            m = dict(wm_g)
            sl = slice((i - 4) * R2, (i - 3) * R2)
            m["xq"] = np.ascontiguousarray(graph16[sl])
            m["kvt"] = np.ascontiguousarray(seqT[:, :, sl])
        in_maps.append(m)
    return in_maps, flags


def kernel(**inputs):
    in_maps, flags = _host_prep(inputs)
    key = tuple(sorted(flags.items()))
    if key not in _cache:
        _cache[key] = _build(flags)
    nc = _cache[key]
    res = run_bass_kernel_spmd(nc, in_maps, core_ids=list(range(N_CORES)))
    seq_out = np.concatenate([res.results[i]["out"] for i in range(4)], axis=0)
    graph_out = np.concatenate([res.results[i]["out"] for i in range(4, 8)],
                               axis=0)
    return (seq_out, graph_out)


# revision 14
# speedup vs baseline: 1.2842x; 1.0840x over previous
"""Trainium2 Bass kernel for nn_BimodalCrossAttentionBlock.

Math: seq-len-1 multihead cross attention => softmax over a single key is
identically 1, so MHA(x_q, x_kv) collapses to out_proj(v_proj(x_kv)) and the
two projections fold into one matrix Wc = out_w @ in_w[2D:] (Q/K projections
and num_heads are dead).  The block then is:
  graph_res = LN(graph + seq @ Wc_s2g.T + bc_s2g)     (gn1)
  seq_res   = LN(seq + graph @ Wc_g2s.T + bc_g2s)     (sn1)
  seq_out   = LN(seq_res + FFN_seq(seq_res))          (sn2)
  graph_out = LN(graph_res + FFN_gr(graph_res))       (gn2)

Sharding: modality-split data parallel.  Cores 0-3 compute seq_out for 8192
rows each; cores 4-7 compute graph_out for 8192 rows each.  Each core then
needs only ONE modality's weights (folded Wc 2MB + FFN w1/w2 16MB fp16),
which fits in SBUF alongside working tiles, so the whole block runs as one
fused, software-pipelined loop: attention matmul -> +residual -> LN1 ->
PE transpose -> FFN(w1/gelu/w2) -> +residual -> LN2 -> out.  No DRAM
round-trip for intermediates and no phase barriers, keeping the PE
continuously busy (TRN2 PE p-states make idle gaps extra costly).  The
kv-side operand is pre-transposed on the host (free) so the PE does no
fp32 input transposes.  Matmuls run fp16 with fp32 PSUM accumulation;
LayerNorm in fp32 (rsqrt via Newton iteration on DVE).
"""
import numpy as np

import concourse.bass as bass
import concourse.bacc as bacc
import concourse.tile as tile
import concourse.mybir as mybir
from concourse.bass_utils import run_bass_kernel_spmd
from concourse.masks import make_identity

F16 = mybir.dt.float16
F32 = mybir.dt.float32
U32 = mybir.dt.uint32
AF = mybir.ActivationFunctionType
ALU = mybir.AluOpType

N_CORES = 8
B_FULL = 32768
D = 1024
HID = 4096
R2 = B_FULL // 4      # rows per core (modality-split: 4 cores per modality)
NB = R2 // 256        # 256-row blocks per core
EPS = 1e-5
MAGIC = 0x5F3759DF

_cache = {}


def _ln_tail(nc, work, magic, x2, out_tile, lng_bc, lnb_bc):
    """LayerNorm of x2 [128, D] f32 -> out_tile; stats + rsqrt all on DVE."""
    stats = work.tile([128, 2, 6], F32, tag="lnstats")
    mv = work.tile([128, 2], F32, tag="lnmv")
    nc.vector.bn_stats(out=stats[:, 0, :], in_=x2[:, 0:512])
    nc.vector.bn_stats(out=stats[:, 1, :], in_=x2[:, 512:1024])
    nc.vector.bn_aggr(out=mv, in_=stats)
    v = work.tile([128, 1], F32, tag="lnv")
    nc.vector.tensor_scalar(out=v, in0=mv[:, 1:2], scalar1=EPS, scalar2=None,
                            op0=ALU.add)
    y = work.tile([128, 1], F32, tag="lny")
    t = work.tile([128, 1], F32, tag="lnt")
    nc.vector.tensor_scalar(out=y.bitcast(U32), in0=v.bitcast(U32), scalar1=1,
                            scalar2=None, op0=ALU.logical_shift_right)
    nc.vector.tensor_tensor(out=y.bitcast(U32), in0=magic, in1=y.bitcast(U32),
                            op=ALU.subtract)
    for _ in range(3):
        nc.vector.tensor_mul(out=t, in0=y, in1=y)
        nc.vector.tensor_mul(out=t, in0=t, in1=v)
        nc.vector.tensor_scalar(out=t, in0=t, scalar1=-0.5, scalar2=1.5,
                                op0=ALU.mult, op1=ALU.add)
        nc.vector.tensor_mul(out=y, in0=y, in1=t)
    if lng_bc is None and lnb_bc is None:
        nc.vector.tensor_scalar(out=out_tile, in0=x2, scalar1=mv[:, 0:1],
                                scalar2=y, op0=ALU.subtract, op1=ALU.mult)
    else:
        tmp = work.tile([128, 1024], F32, tag="lntmp")
        nc.vector.tensor_scalar(out=tmp, in0=x2, scalar1=mv[:, 0:1],
                                scalar2=y, op0=ALU.subtract, op1=ALU.mult)
        if lng_bc is not None:
            nc.vector.tensor_mul(out=tmp, in0=tmp, in1=lng_bc)
        if lnb_bc is not None:
            nc.vector.tensor_add(out=out_tile, in0=tmp, in1=lnb_bc)
        else:
            nc.vector.tensor_copy(out=out_tile, in_=tmp)


def _bcast_param(nc, pool, dram_ap, n, tag):
    t = pool.tile([128, n], F32, tag=tag)
    src = bass.AP(tensor=dram_ap.tensor, offset=dram_ap.offset,
                  ap=[[0, 128]] + dram_ap.ap)
    nc.gpsimd.dma_start(out=t, in_=src)
    return t


def _build(flags):
    fl = lambda k: bool(flags.get(k, False))
    nc = bacc.Bacc("TRN2", target_bir_lowering=False, debug=False,
                   num_devices=N_CORES)

    xq_d = nc.declare_dram_parameter("xq", [R2, D], F16, isOutput=False)
    kvt_d = nc.declare_dram_parameter("kvt", [128, 8, R2], F16, isOutput=False)
    wc_d = nc.declare_dram_parameter("wc", [128, 8, D], F16, isOutput=False)
    w1_d = nc.declare_dram_parameter("w1", [128, 8, HID], F16, isOutput=False)
    w2_d = nc.declare_dram_parameter("w2", [128, 32, D], F16, isOutput=False)
    opt = {}
    for nm, shape, dt in [("bc", [1, D], F16), ("b1", [128, 32], F32),
                          ("b2", [1, D], F16),
                          ("ln1_g", [D], F32), ("ln1_b", [D], F32),
                          ("ln2_g", [D], F32), ("ln2_b", [D], F32)]:
        if fl(nm):
            opt[nm] = nc.declare_dram_parameter(nm, shape, dt, isOutput=False)
    out_d = nc.declare_dram_parameter("out", [R2, D], F32, isOutput=True)

    with tile.TileContext(nc) as tc:
        with tc.tile_pool(name="singles", bufs=1) as singles, \
             tc.tile_pool(name="work", bufs=2) as work, \
             tc.tile_pool(name="lnw", bufs=4) as lnw, \
             tc.tile_pool(name="x2p", bufs=3) as x2p, \
             tc.tile_pool(name="hgp", bufs=8) as hgp, \
             tc.tile_pool(name="psA", bufs=4, space="PSUM") as psA, \
             tc.tile_pool(name="pso", bufs=4, space="PSUM") as pso:

            state = {}

            # ---- block-0 input DMAs first so attention can start early ----
            def load_block(i):
                if i in state:
                    return
                row = i * 256
                xq_t = work.tile([128, 2, D], F16, tag="xq")
                nc.sync.dma_start(
                    out=xq_t,
                    in_=xq_d[row:row + 256, :].rearrange("(s p) n -> p s n", p=128))
                kv_t = work.tile([128, 8, 256], F16, tag="kvt")
                nc.sync.dma_start(out=kv_t, in_=kvt_d[:, :, row:row + 256])
                state[i] = {"xq": xq_t, "kv": kv_t}

            # block 0: interleave kv chunks with wc chunks so the first
            # attention matmul (needs kv chunk kt + wc chunk kt) starts after
            # ~300KB of DMA instead of the full 2.5MB; xq deferred (only
            # needed for the residual add after the matmuls).
            wc_sb = singles.tile([128, 8, D], F16)
            kv0_t = work.tile([128, 8, 256], F16, tag="kvt")
            for kt in range(8):
                nc.sync.dma_start(out=kv0_t[:, kt, :], in_=kvt_d[:, kt, 0:256])
                nc.sync.dma_start(out=wc_sb[:, kt, :], in_=wc_d[:, kt, :])
            xq0_t = work.tile([128, 2, D], F16, tag="xq")
            nc.sync.dma_start(
                out=xq0_t,
                in_=xq_d[0:256, :].rearrange("(s p) n -> p s n", p=128))
            state[0] = {"xq": xq0_t, "kv": kv0_t}
            ident16 = singles.tile([128, 128], F16)
            make_identity(nc, ident16)
            magic = singles.tile([128, 1], U32)
            nc.vector.memset(magic, MAGIC)
            ones16 = None
            bc_sb = None
            if fl("bc"):
                ones16 = singles.tile([1, 128], F16)
                nc.vector.memset(ones16, 1.0)
                bc_sb = singles.tile([1, D], F16)
                nc.sync.dma_start(out=bc_sb, in_=opt["bc"][:, :])
            ln_bcs = {}
            for nm in ("ln1_g", "ln1_b", "ln2_g", "ln2_b"):
                if nm in opt:
                    ln_bcs[nm] = _bcast_param(nc, singles, opt[nm].ap(), D, nm)

            # w1/w2 loaded in deadline order: ffn(0) consumes w1 quarter
            # ht//8 and w2 group ht//4 sequentially, so emit chunks in the
            # order the ht loop will first touch them.
            w1_sb = singles.tile([128, 8, HID], F16)
            w2_sb = singles.tile([128, 32, D], F16)

            # w1/w2 stream on the sync HWDGE queue in deadline order (ffn(0)
            # consumes w1 quarter ht//8 and w2 group ht//4 sequentially).
            # Keeping them on the sync queue measured fastest: routing w2
            # through the scalar HWDGE queue cost ~186us, and DMA-crossbar
            # transposes (scalar queue) cost ~630us via pipeline stalls +
            # PE p-state resets.
            def w1q(q):
                for kt in range(8):
                    nc.sync.dma_start(
                        out=w1_sb[:, kt, q * 1024:(q + 1) * 1024],
                        in_=w1_d[:, kt, q * 1024:(q + 1) * 1024])

            def w2g(g):
                nc.sync.dma_start(out=w2_sb[:, g * 4:(g + 1) * 4, :],
                                  in_=w2_d[:, g * 4:(g + 1) * 4, :])
            b1_sb = None
            if fl("b1"):
                b1_sb = singles.tile([128, 32], F32)
                nc.sync.dma_start(out=b1_sb, in_=opt["b1"][:, :])
            b2_sb = None
            if fl("b2"):
                if ones16 is None:
                    ones16 = singles.tile([1, 128], F16)
                    nc.vector.memset(ones16, 1.0)
                b2_sb = singles.tile([1, D], F16)
                nc.sync.dma_start(out=b2_sb, in_=opt["b2"][:, :])

            # ---------------- pipelined emission ----------------
            def emit_attn(i):
                st = state[i]
                xq_t, kv_t = st["xq"], st["kv"]
                res16 = work.tile([128, 2, D], F16, tag="res16")
                for sub in range(2):
                    pa = [psA.tile([128, 512], F32, tag="acc",
                                   name=f"pa{i}_{sub}_{h}") for h in range(2)]
                    for h in range(2):
                        nsl = slice(h * 512, (h + 1) * 512)
                        for kt in range(8):
                            nc.tensor.matmul(pa[h],
                                             lhsT=kv_t[:, kt, sub * 128:(sub + 1) * 128],
                                             rhs=wc_sb[:, kt, nsl],
                                             start=(kt == 0),
                                             stop=(kt == 7 and bc_sb is None))
                        if bc_sb is not None:
                            nc.tensor.matmul(pa[h], lhsT=ones16,
                                             rhs=bc_sb[:, nsl],
                                             start=False, stop=True)
                    x = work.tile([128, D], F32, tag="x")
                    nc.vector.tensor_add(out=x[:, 0:512],
                                         in0=xq_t[:, sub, 0:512], in1=pa[0])
                    nc.vector.tensor_add(out=x[:, 512:1024],
                                         in0=xq_t[:, sub, 512:1024], in1=pa[1])
                    _ln_tail(nc, lnw, magic, x, res16[:, sub, :],
                             ln_bcs.get("ln1_g"), ln_bcs.get("ln1_b"))
                st["res16"] = res16

            def emit_tp(i):
                res16 = state[i]["res16"]
                rT = work.tile([128, 8, 256], F16, tag="rT")
                for sub in range(2):
                    for grp in range(2):
                        tp = psA.tile([128, 512], F32, tag="acc",
                                      name=f"tp{i}_{sub}_{grp}")
                        tp16 = tp.bitcast(F16)
                        for j in range(4):
                            kt = grp * 4 + j
                            nc.tensor.transpose(tp16[:, j * 128:(j + 1) * 128],
                                                res16[:, sub, kt * 128:(kt + 1) * 128],
                                                ident16)
                        nc.vector.tensor_copy(
                            out=rT[:, grp * 4:(grp + 1) * 4,
                                   sub * 128:(sub + 1) * 128],
                            in_=tp16[:, 0:512].rearrange("p (a b) -> p a b",
                                                         b=128))
                state[i]["rT"] = rT

            def emit_ffn(i):
                rT = state[i]["rT"]
                ops = [pso.tile([128, 512], F32, tag="ops",
                                name=f"ops{i}_{h}") for h in range(4)]
                for ht in range(32):
                    hps = psA.tile([128, 512], F32, tag="acc",
                                   name=f"hps{i}_{ht}")
                    for kt in range(8):
                        nc.tensor.matmul(hps[:, 0:256],
                                         lhsT=w1_sb[:, kt, ht * 128:(ht + 1) * 128],
                                         rhs=rT[:, kt, :],
                                         start=(kt == 0), stop=(kt == 7))
                    hg = hgp.tile([128, 256], F16, tag="hg")
                    if b1_sb is not None:
                        nc.scalar.activation(out=hg, in_=hps[:, 0:256],
                                             func=AF.Gelu,
                                             bias=b1_sb[:, ht:ht + 1],
                                             scale=1.0, alpha=0.0)
                    else:
                        nc.scalar.activation(out=hg, in_=hps[:, 0:256],
                                             func=AF.Gelu)
                    for bs in range(2):
                        for nh in range(2):
                            nc.tensor.matmul(
                                ops[bs * 2 + nh],
                                lhsT=hg[:, bs * 128:(bs + 1) * 128],
                                rhs=w2_sb[:, ht, nh * 512:(nh + 1) * 512],
                                start=(ht == 0),
                                stop=(ht == 31 and b2_sb is None))
                if b2_sb is not None:
                    for bs in range(2):
                        for nh in range(2):
                            nc.tensor.matmul(ops[bs * 2 + nh], lhsT=ones16,
                                             rhs=b2_sb[:, nh * 512:(nh + 1) * 512],
                                             start=False, stop=True)
                state[i]["ops"] = ops

            def emit_out(i):
                res16 = state[i]["res16"]
                ops = state[i]["ops"]
                row = i * 256
                for bs in range(2):
                    x2 = x2p.tile([128, D], F32, tag="x2")
                    nc.vector.tensor_add(out=x2[:, 0:512],
                                         in0=res16[:, bs, 0:512],
                                         in1=ops[bs * 2 + 0])
                    nc.vector.tensor_add(out=x2[:, 512:1024],
                                         in0=res16[:, bs, 512:1024],
                                         in1=ops[bs * 2 + 1])
                    _ln_tail(nc, lnw, magic, x2, x2,
                             ln_bcs.get("ln2_g"), ln_bcs.get("ln2_b"))
                    nc.sync.dma_start(
                        out=out_d[row + bs * 128:row + bs * 128 + 128, :],
                        in_=x2)
                del state[i]

            # last block runs as two 128-row FFN passes so the end-of-kernel
            # drain (adds+LN2+DMA after the final w2 matmul) covers 128 rows
            # instead of 256; the first half's output path overlaps the
            # second half's FFN.
            def emit_ffn_half(i, half):
                rT = state[i]["rT"]
                ops = [pso.tile([128, 512], F32, tag="ops",
                                name=f"opsh{i}_{half}_{h}") for h in range(2)]
                for ht in range(32):
                    hps = psA.tile([128, 512], F32, tag="acc",
                                   name=f"hpsh{i}_{half}_{ht}")
                    for kt in range(8):
                        nc.tensor.matmul(hps[:, 0:128],
                                         lhsT=w1_sb[:, kt, ht * 128:(ht + 1) * 128],
                                         rhs=rT[:, kt, half * 128:(half + 1) * 128],
                                         start=(kt == 0), stop=(kt == 7))
                    hg = hgp.tile([128, 256], F16, tag="hg")
                    if b1_sb is not None:
                        nc.scalar.activation(out=hg[:, 0:128], in_=hps[:, 0:128],
                                             func=AF.Gelu,
                                             bias=b1_sb[:, ht:ht + 1],
                                             scale=1.0, alpha=0.0)
                    else:
                        nc.scalar.activation(out=hg[:, 0:128], in_=hps[:, 0:128],
                                             func=AF.Gelu)
                    for nh in range(2):
                        nc.tensor.matmul(ops[nh], lhsT=hg[:, 0:128],
                                         rhs=w2_sb[:, ht, nh * 512:(nh + 1) * 512],
                                         start=(ht == 0),
                                         stop=(ht == 31 and b2_sb is None))
                if b2_sb is not None:
                    for nh in range(2):
                        nc.tensor.matmul(ops[nh], lhsT=ones16,
                                         rhs=b2_sb[:, nh * 512:(nh + 1) * 512],
                                         start=False, stop=True)
                return ops

            def emit_out_half(i, half, ops):
                res16 = state[i]["res16"]
                row = i * 256 + half * 128
                x2 = x2p.tile([128, D], F32, tag="x2")
                nc.vector.tensor_add(out=x2[:, 0:512],
                                     in0=res16[:, half, 0:512], in1=ops[0])
                nc.vector.tensor_add(out=x2[:, 512:1024],
                                     in0=res16[:, half, 512:1024], in1=ops[1])
                _ln_tail(nc, lnw, magic, x2, x2,
                         ln_bcs.get("ln2_g"), ln_bcs.get("ln2_b"))
                nc.sync.dma_start(out=out_d[row:row + 128, :], in_=x2)

            emit_attn(0)
            emit_tp(0)
            w1q(0)
            w2g(0)
            load_block(1)
            w2g(1)
            w1q(1)
            w2g(2)
            w2g(3)
            w1q(2)
            w2g(4)
            w2g(5)
            w1q(3)
            w2g(6)
            w2g(7)
            for i in range(NB):
                if i + 1 < NB:
                    load_block(i + 1)
                    emit_attn(i + 1)
                if i == NB - 1:
                    opsA = emit_ffn_half(i, 0)
                    emit_out_half(i, 0, opsA)
                    opsB = emit_ffn_half(i, 1)
                    emit_out_half(i, 1, opsB)
                    del state[i]
                else:
                    emit_ffn(i)
                    emit_tp(i + 1)
                    emit_out(i)

    nc.compile()
    return nc


def _host_prep(inputs):
    f = lambda k: np.asarray(inputs[k])
    flags = {}

    def fold(pfx):
        in_w = f(f"{pfx}_in_w").astype(np.float64)
        in_b = f(f"{pfx}_in_b").astype(np.float64)
        out_w = f(f"{pfx}_out_w").astype(np.float64)
        out_b = f(f"{pfx}_out_b").astype(np.float64)
        Wc = out_w @ in_w[2 * D:]
        bc = in_b[2 * D:] @ out_w.T + out_b
        return Wc, bc

    Wcs, bcs = fold("s2g")   # kv = seq, updates graph
    Wcg, bcg = fold("g2s")   # kv = graph, updates seq

    def rhs_tiles(W, kt):  # W [n, d_in] -> [128, kt, n] f16 tiles of W.T
        return np.ascontiguousarray(
            W.T.reshape(kt, 128, -1).transpose(1, 0, 2)).astype(np.float16)

    def t_tiles(X):  # X [B, D] -> [128, 8, B] f16 tiles of X.T
        return np.ascontiguousarray(
            X.T.reshape(8, 128, -1).transpose(1, 0, 2)).astype(np.float16)

    seq = f("seq_emb").astype(np.float32)
    graph = f("graph_emb").astype(np.float32)
    seqT = t_tiles(seq)
    graphT = t_tiles(graph)
    seq16 = seq.astype(np.float16)
    graph16 = graph.astype(np.float16)

    # flags are the union over both modalities (one SPMD program for all
    # cores); zero/identity values are passed where a modality's param is
    # trivial.
    flags_probe = {
        "bc": np.any(bcs != 0) or np.any(bcg != 0),
        "b1": np.any(f("seq_b1") != 0) or np.any(f("gr_b1") != 0),
        "b2": np.any(f("seq_b2") != 0) or np.any(f("gr_b2") != 0),
        "ln1_g": np.any(f("sn1_g") != 1) or np.any(f("gn1_g") != 1),
        "ln1_b": np.any(f("sn1_b") != 0) or np.any(f("gn1_b") != 0),
        "ln2_g": np.any(f("sn2_g") != 1) or np.any(f("gn2_g") != 1),
        "ln2_b": np.any(f("sn2_b") != 0) or np.any(f("gn2_b") != 0),
    }
    for k, v in flags_probe.items():
        if v:
            flags[k] = True

    def modality_map(wc, bc, w1, b1, w2, b2, ln1g, ln1b, ln2g, ln2b):
        m = {"wc": rhs_tiles(wc, 8), "w1": rhs_tiles(w1, 8),
             "w2": rhs_tiles(w2, 32)}
        if "bc" in flags:
            m["bc"] = bc.astype(np.float16).reshape(1, D)
        if "b1" in flags:
            m["b1"] = np.ascontiguousarray(
                b1.reshape(32, 128).T).astype(np.float32)
        if "b2" in flags:
            m["b2"] = b2.astype(np.float16).reshape(1, D)
        for nm, v, dflt in (("ln1_g", ln1g, 1.0), ("ln1_b", ln1b, 0.0),
                            ("ln2_g", ln2g, 1.0), ("ln2_b", ln2b, 0.0)):
            if nm in flags:
                m[nm] = np.asarray(v, dtype=np.float32)
        return m

    # seq cores: xq = seq, kv = graph, wc = Wcg (g2s), FFN = seq_*
    wm_s = modality_map(Wcg, bcg, f("seq_w1"), f("seq_b1"), f("seq_w2"),
                        f("seq_b2"), f("sn1_g"), f("sn1_b"), f("sn2_g"),
                        f("sn2_b"))
    # graph cores: xq = graph, kv = seq, wc = Wcs (s2g), FFN = gr_*
    wm_g = modality_map(Wcs, bcs, f("gr_w1"), f("gr_b1"), f("gr_w2"),
                        f("gr_b2"), f("gn1_g"), f("gn1_b"), f("gn2_g"),
                        f("gn2_b"))

    in_maps = []
    for i in range(N_CORES):
        if i < 4:
            m = dict(wm_s)
            sl = slice(i * R2, (i + 1) * R2)
            m["xq"] = np.ascontiguousarray(seq16[sl])
            m["kvt"] = np.ascontiguousarray(graphT[:, :, sl])
        else:
            m = dict(wm_g)
            sl = slice((i - 4) * R2, (i - 3) * R2)
            m["xq"] = np.ascontiguousarray(graph16[sl])
            m["kvt"] = np.ascontiguousarray(seqT[:, :, sl])
        in_maps.append(m)
    return in_maps, flags


def kernel(**inputs):
    in_maps, flags = _host_prep(inputs)
    key = tuple(sorted(flags.items()))
    if key not in _cache:
        _cache[key] = _build(flags)
    nc = _cache[key]
    res = run_bass_kernel_spmd(nc, in_maps, core_ids=list(range(N_CORES)))
    seq_out = np.concatenate([res.results[i]["out"] for i in range(4)], axis=0)
    graph_out = np.concatenate([res.results[i]["out"] for i in range(4, 8)],
                               axis=0)
    return (seq_out, graph_out)


# revision 19
# speedup vs baseline: 1.2843x; 1.0001x over previous
"""Trainium2 Bass kernel for nn_BimodalCrossAttentionBlock.

Math: seq-len-1 multihead cross attention => softmax over a single key is
identically 1, so MHA(x_q, x_kv) collapses to out_proj(v_proj(x_kv)) and the
two projections fold into one matrix Wc = out_w @ in_w[2D:] (Q/K projections
and num_heads are dead).  The block then is:
  graph_res = LN(graph + seq @ Wc_s2g.T + bc_s2g)     (gn1)
  seq_res   = LN(seq + graph @ Wc_g2s.T + bc_g2s)     (sn1)
  seq_out   = LN(seq_res + FFN_seq(seq_res))          (sn2)
  graph_out = LN(graph_res + FFN_gr(graph_res))       (gn2)

Sharding: modality-split data parallel.  Cores 0-3 compute seq_out for 8192
rows each; cores 4-7 compute graph_out for 8192 rows each.  Each core then
needs only ONE modality's weights (folded Wc 2MB + FFN w1/w2 16MB fp16),
which fits in SBUF alongside working tiles, so the whole block runs as one
fused, software-pipelined loop: attention matmul -> +residual -> LN1 ->
PE transpose -> FFN(w1/gelu/w2) -> +residual -> LN2 -> out.  No DRAM
round-trip for intermediates and no phase barriers, keeping the PE
continuously busy (TRN2 PE p-states make idle gaps extra costly).  The
kv-side operand is pre-transposed on the host (free) so the PE does no
fp32 input transposes.  Matmuls run fp16 with fp32 PSUM accumulation;
LayerNorm in fp32 (rsqrt via Newton iteration on DVE).
"""
import numpy as np

import concourse.bass as bass
import concourse.bacc as bacc
import concourse.tile as tile
import concourse.mybir as mybir
from concourse.bass_utils import run_bass_kernel_spmd
from concourse.masks import make_identity

F16 = mybir.dt.float16
F32 = mybir.dt.float32
U32 = mybir.dt.uint32
AF = mybir.ActivationFunctionType
ALU = mybir.AluOpType

N_CORES = 8
B_FULL = 32768
D = 1024
HID = 4096
R2 = B_FULL // 4      # rows per core (modality-split: 4 cores per modality)
NB = R2 // 256        # 256-row blocks per core
EPS = 1e-5
MAGIC = 0x5F3759DF

_cache = {}


def _ln_tail(nc, work, magic, x2, out_tile, lng_bc, lnb_bc):
    """LayerNorm of x2 [128, D] f32 -> out_tile; stats + rsqrt all on DVE."""
    stats = work.tile([128, 2, 6], F32, tag="lnstats")
    mv = work.tile([128, 2], F32, tag="lnmv")
    nc.vector.bn_stats(out=stats[:, 0, :], in_=x2[:, 0:512])
    nc.vector.bn_stats(out=stats[:, 1, :], in_=x2[:, 512:1024])
    nc.vector.bn_aggr(out=mv, in_=stats)
    v = work.tile([128, 1], F32, tag="lnv")
    nc.vector.tensor_scalar(out=v, in0=mv[:, 1:2], scalar1=EPS, scalar2=None,
                            op0=ALU.add)
    y = work.tile([128, 1], F32, tag="lny")
    t = work.tile([128, 1], F32, tag="lnt")
    nc.vector.tensor_scalar(out=y.bitcast(U32), in0=v.bitcast(U32), scalar1=1,
                            scalar2=None, op0=ALU.logical_shift_right)
    nc.vector.tensor_tensor(out=y.bitcast(U32), in0=magic, in1=y.bitcast(U32),
                            op=ALU.subtract)
    for _ in range(3):
        nc.vector.tensor_mul(out=t, in0=y, in1=y)
        nc.vector.tensor_mul(out=t, in0=t, in1=v)
        nc.vector.tensor_scalar(out=t, in0=t, scalar1=-0.5, scalar2=1.5,
                                op0=ALU.mult, op1=ALU.add)
        nc.vector.tensor_mul(out=y, in0=y, in1=t)
    if lng_bc is None and lnb_bc is None:
        nc.vector.tensor_scalar(out=out_tile, in0=x2, scalar1=mv[:, 0:1],
                                scalar2=y, op0=ALU.subtract, op1=ALU.mult)
    else:
        tmp = work.tile([128, 1024], F32, tag="lntmp")
        nc.vector.tensor_scalar(out=tmp, in0=x2, scalar1=mv[:, 0:1],
                                scalar2=y, op0=ALU.subtract, op1=ALU.mult)
        if lng_bc is not None:
            nc.vector.tensor_mul(out=tmp, in0=tmp, in1=lng_bc)
        if lnb_bc is not None:
            nc.vector.tensor_add(out=out_tile, in0=tmp, in1=lnb_bc)
        else:
            nc.vector.tensor_copy(out=out_tile, in_=tmp)


def _bcast_param(nc, pool, dram_ap, n, tag):
    t = pool.tile([128, n], F32, tag=tag)
    src = bass.AP(tensor=dram_ap.tensor, offset=dram_ap.offset,
                  ap=[[0, 128]] + dram_ap.ap)
    nc.gpsimd.dma_start(out=t, in_=src)
    return t


def _build(flags):
    fl = lambda k: bool(flags.get(k, False))
    nc = bacc.Bacc("TRN2", target_bir_lowering=False, debug=False,
                   num_devices=N_CORES)

    xq_d = nc.declare_dram_parameter("xq", [R2, D], F16, isOutput=False)
    # kvt is block-major and w1 quarter-major so the startup/streaming DMAs
    # read long contiguous runs (4-16KB) instead of 0.5-2KB strided lines.
    kvt_d = nc.declare_dram_parameter("kvt", [NB, 128, 8, 256], F16,
                                      isOutput=False)
    wc_d = nc.declare_dram_parameter("wc", [128, 8, D], F16, isOutput=False)
    w1_d = nc.declare_dram_parameter("w1", [4, 128, 8, HID // 4], F16,
                                     isOutput=False)
    w2_d = nc.declare_dram_parameter("w2", [128, 32, D], F16, isOutput=False)
    opt = {}
    for nm, shape, dt in [("bc", [1, D], F16), ("b1", [128, 32], F32),
                          ("b2", [1, D], F16),
                          ("ln1_g", [D], F32), ("ln1_b", [D], F32),
                          ("ln2_g", [D], F32), ("ln2_b", [D], F32)]:
        if fl(nm):
            opt[nm] = nc.declare_dram_parameter(nm, shape, dt, isOutput=False)
    out_d = nc.declare_dram_parameter("out", [R2, D], F32, isOutput=True)

    with tile.TileContext(nc) as tc:
        with tc.tile_pool(name="singles", bufs=1) as singles, \
             tc.tile_pool(name="work", bufs=2) as work, \
             tc.tile_pool(name="lnw", bufs=4) as lnw, \
             tc.tile_pool(name="x2p", bufs=3) as x2p, \
             tc.tile_pool(name="hgp", bufs=8) as hgp, \
             tc.tile_pool(name="psA", bufs=4, space="PSUM") as psA, \
             tc.tile_pool(name="pso", bufs=4, space="PSUM") as pso:

            state = {}

            # ---- block-0 input DMAs first so attention can start early ----
            def load_block(i):
                if i in state:
                    return
                row = i * 256
                xq_t = work.tile([128, 2, D], F16, tag="xq")
                nc.sync.dma_start(
                    out=xq_t,
                    in_=xq_d[row:row + 256, :].rearrange("(s p) n -> p s n", p=128))
                kv_t = work.tile([128, 8, 256], F16, tag="kvt")
                nc.sync.dma_start(out=kv_t, in_=kvt_d[i])
                state[i] = {"xq": xq_t, "kv": kv_t}

            # block 0: kv tile first (one contiguous 0.25MB transfer), then
            # wc in per-kt chunks so the first attention matmul starts early;
            # xq deferred (only needed for the residual add after the mms).
            wc_sb = singles.tile([128, 8, D], F16)
            kv0_t = work.tile([128, 8, 256], F16, tag="kvt")
            nc.sync.dma_start(out=kv0_t, in_=kvt_d[0])
            for kt in range(8):
                nc.sync.dma_start(out=wc_sb[:, kt, :], in_=wc_d[:, kt, :])
            xq0_t = work.tile([128, 2, D], F16, tag="xq")
            nc.sync.dma_start(
                out=xq0_t,
                in_=xq_d[0:256, :].rearrange("(s p) n -> p s n", p=128))
            state[0] = {"xq": xq0_t, "kv": kv0_t}
            ident16 = singles.tile([128, 128], F16)
            make_identity(nc, ident16)
            magic = singles.tile([128, 1], U32)
            nc.vector.memset(magic, MAGIC)
            ones16 = None
            bc_sb = None
            if fl("bc"):
                ones16 = singles.tile([1, 128], F16)
                nc.vector.memset(ones16, 1.0)
                bc_sb = singles.tile([1, D], F16)
                nc.sync.dma_start(out=bc_sb, in_=opt["bc"][:, :])
            ln_bcs = {}
            for nm in ("ln1_g", "ln1_b", "ln2_g", "ln2_b"):
                if nm in opt:
                    ln_bcs[nm] = _bcast_param(nc, singles, opt[nm].ap(), D, nm)

            # w1/w2 loaded in deadline order: ffn(0) consumes w1 quarter
            # ht//8 and w2 group ht//4 sequentially, so emit chunks in the
            # order the ht loop will first touch them.
            w1_sb = singles.tile([128, 8, HID], F16)
            w2_sb = singles.tile([128, 32, D], F16)

            # w1/w2 stream on the sync HWDGE queue in deadline order (ffn(0)
            # consumes w1 quarter ht//8 and w2 group ht//4 sequentially).
            # Keeping them on the sync queue measured fastest: routing w2
            # through the scalar HWDGE queue cost ~186us, and DMA-crossbar
            # transposes (scalar queue) cost ~630us via pipeline stalls +
            # PE p-state resets.
            def w1q(q):
                nc.sync.dma_start(out=w1_sb[:, :, q * 1024:(q + 1) * 1024],
                                  in_=w1_d[q])

            def w2g(g):
                nc.sync.dma_start(out=w2_sb[:, g * 4:(g + 1) * 4, :],
                                  in_=w2_d[:, g * 4:(g + 1) * 4, :])
            b1_sb = None
            if fl("b1"):
                b1_sb = singles.tile([128, 32], F32)
                nc.sync.dma_start(out=b1_sb, in_=opt["b1"][:, :])
            b2_sb = None
            if fl("b2"):
                if ones16 is None:
                    ones16 = singles.tile([1, 128], F16)
                    nc.vector.memset(ones16, 1.0)
                b2_sb = singles.tile([1, D], F16)
                nc.sync.dma_start(out=b2_sb, in_=opt["b2"][:, :])

            # ---------------- pipelined emission ----------------
            def emit_attn(i):
                st = state[i]
                xq_t, kv_t = st["xq"], st["kv"]
                res16 = work.tile([128, 2, D], F16, tag="res16")
                for sub in range(2):
                    pa = [psA.tile([128, 512], F32, tag="acc",
                                   name=f"pa{i}_{sub}_{h}") for h in range(2)]
                    for h in range(2):
                        nsl = slice(h * 512, (h + 1) * 512)
                        for kt in range(8):
                            nc.tensor.matmul(pa[h],
                                             lhsT=kv_t[:, kt, sub * 128:(sub + 1) * 128],
                                             rhs=wc_sb[:, kt, nsl],
                                             start=(kt == 0),
                                             stop=(kt == 7 and bc_sb is None))
                        if bc_sb is not None:
                            nc.tensor.matmul(pa[h], lhsT=ones16,
                                             rhs=bc_sb[:, nsl],
                                             start=False, stop=True)
                    x = work.tile([128, D], F32, tag="x")
                    nc.vector.tensor_add(out=x[:, 0:512],
                                         in0=xq_t[:, sub, 0:512], in1=pa[0])
                    nc.vector.tensor_add(out=x[:, 512:1024],
                                         in0=xq_t[:, sub, 512:1024], in1=pa[1])
                    _ln_tail(nc, lnw, magic, x, res16[:, sub, :],
                             ln_bcs.get("ln1_g"), ln_bcs.get("ln1_b"))
                st["res16"] = res16

            def emit_tp(i):
                res16 = state[i]["res16"]
                rT = work.tile([128, 8, 256], F16, tag="rT")
                for sub in range(2):
                    for grp in range(2):
                        tp = psA.tile([128, 512], F32, tag="acc",
                                      name=f"tp{i}_{sub}_{grp}")
                        tp16 = tp.bitcast(F16)
                        for j in range(4):
                            kt = grp * 4 + j
                            nc.tensor.transpose(tp16[:, j * 128:(j + 1) * 128],
                                                res16[:, sub, kt * 128:(kt + 1) * 128],
                                                ident16)
                        nc.vector.tensor_copy(
                            out=rT[:, grp * 4:(grp + 1) * 4,
                                   sub * 128:(sub + 1) * 128],
                            in_=tp16[:, 0:512].rearrange("p (a b) -> p a b",
                                                         b=128))
                state[i]["rT"] = rT

            def emit_ffn(i):
                rT = state[i]["rT"]
                ops = [pso.tile([128, 512], F32, tag="ops",
                                name=f"ops{i}_{h}") for h in range(4)]
                for ht in range(32):
                    hps = psA.tile([128, 512], F32, tag="acc",
                                   name=f"hps{i}_{ht}")
                    for kt in range(8):
                        nc.tensor.matmul(hps[:, 0:256],
                                         lhsT=w1_sb[:, kt, ht * 128:(ht + 1) * 128],
                                         rhs=rT[:, kt, :],
                                         start=(kt == 0), stop=(kt == 7))
                    hg = hgp.tile([128, 256], F16, tag="hg")
                    if b1_sb is not None:
                        nc.scalar.activation(out=hg, in_=hps[:, 0:256],
                                             func=AF.Gelu,
                                             bias=b1_sb[:, ht:ht + 1],
                                             scale=1.0, alpha=0.0)
                    else:
                        nc.scalar.activation(out=hg, in_=hps[:, 0:256],
                                             func=AF.Gelu)
                    for bs in range(2):
                        for nh in range(2):
                            nc.tensor.matmul(
                                ops[bs * 2 + nh],
                                lhsT=hg[:, bs * 128:(bs + 1) * 128],
                                rhs=w2_sb[:, ht, nh * 512:(nh + 1) * 512],
                                start=(ht == 0),
                                stop=(ht == 31 and b2_sb is None))
                if b2_sb is not None:
                    for bs in range(2):
                        for nh in range(2):
                            nc.tensor.matmul(ops[bs * 2 + nh], lhsT=ones16,
                                             rhs=b2_sb[:, nh * 512:(nh + 1) * 512],
                                             start=False, stop=True)
                state[i]["ops"] = ops

            def emit_out(i):
                res16 = state[i]["res16"]
                ops = state[i]["ops"]
                row = i * 256
                for bs in range(2):
                    x2 = x2p.tile([128, D], F32, tag="x2")
                    nc.vector.tensor_add(out=x2[:, 0:512],
                                         in0=res16[:, bs, 0:512],
                                         in1=ops[bs * 2 + 0])
                    nc.vector.tensor_add(out=x2[:, 512:1024],
                                         in0=res16[:, bs, 512:1024],
                                         in1=ops[bs * 2 + 1])
                    _ln_tail(nc, lnw, magic, x2, x2,
                             ln_bcs.get("ln2_g"), ln_bcs.get("ln2_b"))
                    nc.sync.dma_start(
                        out=out_d[row + bs * 128:row + bs * 128 + 128, :],
                        in_=x2)
                del state[i]

            # last block runs as two 128-row FFN passes so the end-of-kernel
            # drain (adds+LN2+DMA after the final w2 matmul) covers 128 rows
            # instead of 256; the first half's output path overlaps the
            # second half's FFN.
            def emit_ffn_half(i, half):
                rT = state[i]["rT"]
                ops = [pso.tile([128, 512], F32, tag="ops",
                                name=f"opsh{i}_{half}_{h}") for h in range(2)]
                for ht in range(32):
                    hps = psA.tile([128, 512], F32, tag="acc",
                                   name=f"hpsh{i}_{half}_{ht}")
                    for kt in range(8):
                        nc.tensor.matmul(hps[:, 0:128],
                                         lhsT=w1_sb[:, kt, ht * 128:(ht + 1) * 128],
                                         rhs=rT[:, kt, half * 128:(half + 1) * 128],
                                         start=(kt == 0), stop=(kt == 7))
                    hg = hgp.tile([128, 256], F16, tag="hg")
                    if b1_sb is not None:
                        nc.scalar.activation(out=hg[:, 0:128], in_=hps[:, 0:128],
                                             func=AF.Gelu,
                                             bias=b1_sb[:, ht:ht + 1],
                                             scale=1.0, alpha=0.0)
                    else:
                        nc.scalar.activation(out=hg[:, 0:128], in_=hps[:, 0:128],
                                             func=AF.Gelu)
                    for nh in range(2):
                        nc.tensor.matmul(ops[nh], lhsT=hg[:, 0:128],
                                         rhs=w2_sb[:, ht, nh * 512:(nh + 1) * 512],
                                         start=(ht == 0),
                                         stop=(ht == 31 and b2_sb is None))
                if b2_sb is not None:
                    for nh in range(2):
                        nc.tensor.matmul(ops[nh], lhsT=ones16,
                                         rhs=b2_sb[:, nh * 512:(nh + 1) * 512],
                                         start=False, stop=True)
                return ops

            def emit_out_half(i, half, ops):
                res16 = state[i]["res16"]
                row = i * 256 + half * 128
                x2 = x2p.tile([128, D], F32, tag="x2")
                nc.vector.tensor_add(out=x2[:, 0:512],
                                     in0=res16[:, half, 0:512], in1=ops[0])
                nc.vector.tensor_add(out=x2[:, 512:1024],
                                     in0=res16[:, half, 512:1024], in1=ops[1])
                _ln_tail(nc, lnw, magic, x2, x2,
                         ln_bcs.get("ln2_g"), ln_bcs.get("ln2_b"))
                nc.sync.dma_start(out=out_d[row:row + 128, :], in_=x2)

            emit_attn(0)
            emit_tp(0)
            w1q(0)
            w2g(0)
            load_block(1)
            w2g(1)
            w1q(1)
            w2g(2)
            w2g(3)
            w1q(2)
            w2g(4)
            w2g(5)
            w1q(3)
            w2g(6)
            w2g(7)
            for i in range(NB):
                if i + 1 < NB:
                    load_block(i + 1)
                    emit_attn(i + 1)
                if i == NB - 1:
                    opsA = emit_ffn_half(i, 0)
                    emit_out_half(i, 0, opsA)
                    opsB = emit_ffn_half(i, 1)
                    emit_out_half(i, 1, opsB)
                    del state[i]
                else:
                    emit_ffn(i)
                    emit_tp(i + 1)
                    emit_out(i)

    nc.compile()
    return nc


def _host_prep(inputs):
    f = lambda k: np.asarray(inputs[k])
    flags = {}

    def fold(pfx):
        in_w = f(f"{pfx}_in_w").astype(np.float64)
        in_b = f(f"{pfx}_in_b").astype(np.float64)
        out_w = f(f"{pfx}_out_w").astype(np.float64)
        out_b = f(f"{pfx}_out_b").astype(np.float64)
        Wc = out_w @ in_w[2 * D:]
        bc = in_b[2 * D:] @ out_w.T + out_b
        return Wc, bc

    Wcs, bcs = fold("s2g")   # kv = seq, updates graph
    Wcg, bcg = fold("g2s")   # kv = graph, updates seq

    def rhs_tiles(W, kt):  # W [n, d_in] -> [128, kt, n] f16 tiles of W.T
        return np.ascontiguousarray(
            W.T.reshape(kt, 128, -1).transpose(1, 0, 2)).astype(np.float16)

    def t_tiles(X):  # X [B, D] -> [128, 8, B] f16 tiles of X.T
        return np.ascontiguousarray(
            X.T.reshape(8, 128, -1).transpose(1, 0, 2)).astype(np.float16)

    seq = f("seq_emb").astype(np.float32)
    graph = f("graph_emb").astype(np.float32)
    seqT = t_tiles(seq)
    graphT = t_tiles(graph)
    seq16 = seq.astype(np.float16)
    graph16 = graph.astype(np.float16)

    # flags are the union over both modalities (one SPMD program for all
    # cores); zero/identity values are passed where a modality's param is
    # trivial.
    flags_probe = {
        "bc": np.any(bcs != 0) or np.any(bcg != 0),
        "b1": np.any(f("seq_b1") != 0) or np.any(f("gr_b1") != 0),
        "b2": np.any(f("seq_b2") != 0) or np.any(f("gr_b2") != 0),
        "ln1_g": np.any(f("sn1_g") != 1) or np.any(f("gn1_g") != 1),
        "ln1_b": np.any(f("sn1_b") != 0) or np.any(f("gn1_b") != 0),
        "ln2_g": np.any(f("sn2_g") != 1) or np.any(f("gn2_g") != 1),
        "ln2_b": np.any(f("sn2_b") != 0) or np.any(f("gn2_b") != 0),
    }
    for k, v in flags_probe.items():
        if v:
            flags[k] = True

    def modality_map(wc, bc, w1, b1, w2, b2, ln1g, ln1b, ln2g, ln2b):
        w1t = rhs_tiles(w1, 8)  # [128, 8, 4096]
        m = {"wc": rhs_tiles(wc, 8),
             "w1": np.ascontiguousarray(
                 w1t.reshape(128, 8, 4, 1024).transpose(2, 0, 1, 3)),
             "w2": rhs_tiles(w2, 32)}
        if "bc" in flags:
            m["bc"] = bc.astype(np.float16).reshape(1, D)
        if "b1" in flags:
            m["b1"] = np.ascontiguousarray(
                b1.reshape(32, 128).T).astype(np.float32)
        if "b2" in flags:
            m["b2"] = b2.astype(np.float16).reshape(1, D)
        for nm, v, dflt in (("ln1_g", ln1g, 1.0), ("ln1_b", ln1b, 0.0),
                            ("ln2_g", ln2g, 1.0), ("ln2_b", ln2b, 0.0)):
            if nm in flags:
                m[nm] = np.asarray(v, dtype=np.float32)
        return m

    # seq cores: xq = seq, kv = graph, wc = Wcg (g2s), FFN = seq_*
    wm_s = modality_map(Wcg, bcg, f("seq_w1"), f("seq_b1"), f("seq_w2"),
                        f("seq_b2"), f("sn1_g"), f("sn1_b"), f("sn2_g"),
                        f("sn2_b"))
    # graph cores: xq = graph, kv = seq, wc = Wcs (s2g), FFN = gr_*
    wm_g = modality_map(Wcs, bcs, f("gr_w1"), f("gr_b1"), f("gr_w2"),
                        f("gr_b2"), f("gn1_g"), f("gn1_b"), f("gn2_g"),
                        f("gn2_b"))

    def kv_blocks(T, sl):  # [128, 8, R2] slice -> block-major [NB, 128, 8, 256]
        K = np.ascontiguousarray(T[:, :, sl])
        return np.ascontiguousarray(
            K.reshape(128, 8, R2 // 256, 256).transpose(2, 0, 1, 3))

    in_maps = []
    for i in range(N_CORES):
        if i < 4:
            m = dict(wm_s)
            sl = slice(i * R2, (i + 1) * R2)
            m["xq"] = np.ascontiguousarray(seq16[sl])
            m["kvt"] = kv_blocks(graphT, sl)
        else:
            m = dict(wm_g)
            sl = slice((i - 4) * R2, (i - 3) * R2)
            m["xq"] = np.ascontiguousarray(graph16[sl])
            m["kvt"] = kv_blocks(seqT, sl)
        in_maps.append(m)
    return in_maps, flags


def kernel(**inputs):
    in_maps, flags = _host_prep(inputs)
    key = tuple(sorted(flags.items()))
    if key not in _cache:
        _cache[key] = _build(flags)
    nc = _cache[key]
    res = run_bass_kernel_spmd(nc, in_maps, core_ids=list(range(N_CORES)))
    seq_out = np.concatenate([res.results[i]["out"] for i in range(4)], axis=0)
    graph_out = np.concatenate([res.results[i]["out"] for i in range(4, 8)],
                               axis=0)
    return (seq_out, graph_out)
